# revision 2
# baseline (speedup 1.0000x reference)
import sys

sys.path.insert(0, "/opt/trn_rl_repo")

import numpy as np
import ml_dtypes

import concourse.bass as bass
import concourse.tile as tile
from concourse import bacc, mybir
from concourse.bass_utils import run_bass_kernel_spmd

# Problem constants (hardcoded per contract)
B, N, F = 8, 512, 16
D, PH, PW = 150, 26, 26
IMG = 128
C = 64
HW = PH * PW  # 676

# z-pair table: pair p holds slabs (15+p, 75+p); covers z in [15,135)
NPAIR = 60
ZLO = 15
S1CAP = 32  # stage-1 slots per pair (32-aligned; empty slots hold zeros)
S1PAD = 32

# padded patch storage in DRAM: per stage-1 slot, 26 patch rows + 30 zero
# rows, each ROWPX px with the 26-px piece at cols [PIECE0, PIECE0+26).
ROWPX = 60
PIECE0 = 32
SROWS = 56   # storage rows per slot (26 patch + 30 zeros)
WROWS = 60   # gather window rows; tile t covers canvas rows [26t-4, 26t+56)
ZLEAD = 30   # leading zero rows in storage

# stage-3 canvas tiling: strips of 52 rows at stride 26 (strip t covers
# canvas rows [26t, 26t+52)); x-banks of width 32 by x0.
BANK_LO = (14, 46, 78, 110)
NBANK = 4
NSTRIP = 6
NT3 = NSTRIP * NBANK

# psum-column intervals [a,b) and the x-banks writing each
INTERVALS = (
    (0, 20, (0,)),
    (20, 45, (0, 1)),
    (45, 52, (1,)),
    (52, 77, (1, 2)),
    (77, 84, (2,)),
    (84, 109, (2, 3)),
    (109, 128, (3,)),
)

_compiled = None
_meta_key = None
_rend_zeros = None
DEBUG = False


def _geom(xyz, inv_voxel_size, psf_center):
    u = (xyz * inv_voxel_size).astype(np.float32).copy()
    u[..., :2] -= psf_center[:2]
    u[..., 2] += psf_center[2]
    u_floor = np.floor(u)
    frac = u - u_floor
    ui = u_floor.astype(np.int32)
    x_idx = ui[..., 0] + PW
    y_idx = ui[..., 1] + PH
    z_idx = ui[..., 2]
    frac[..., :2] = 1.0 - frac[..., :2]
    p = frac[..., None] ** np.arange(4, dtype=np.float32)  # (B,N,3,4)
    vx, vy, vz = p[..., 0, :], p[..., 1, :], p[..., 2, :]
    series = (
        vz[..., :, None, None] * vx[..., None, :, None] * vy[..., None, None, :]
    ).reshape(B, N, C)
    return x_idx, y_idx, z_idx, series


def _build_meta(x_idx, y_idx, z_idx):
    half = (z_idx >= 75).astype(np.int32)
    pair = np.where(half == 0, z_idx - ZLO, z_idx - 75)
    assert pair.min() >= 0 and pair.max() < NPAIR

    cap1 = np.zeros(NPAIR, np.int64)
    for b in range(B):
        cap1 = np.maximum(cap1, np.bincount(pair[b], minlength=NPAIR))
    assert cap1.max() <= S1CAP, cap1.max()
    off1 = np.arange(NPAIR, dtype=np.int64) * S1PAD  # lhsT1 column offsets
    s1tot = NPAIR * S1CAP  # DRAM storage slots (32-padded per pair)

    strip = y_idx // 26
    bank = np.clip((x_idx - 14) // 32, 0, NBANK - 1)
    cap3 = np.zeros(NT3, np.int64)
    for b in range(B):
        cnt = np.bincount(strip[b] * NBANK + bank[b], minlength=NT3)
        cap3 = np.maximum(cap3, cnt)
    # pack tiles into 128-partition gather objects
    obj_id = np.zeros(NT3, np.int64)
    pbase = np.zeros(NT3, np.int64)
    o, used = 0, 0
    for ti in range(NT3):
        c = int(cap3[ti])
        if used + c > 128:
            o, used = o + 1, 0
        obj_id[ti] = o
        pbase[ti] = used
        used += c
    nobj = o + 1

    rtot = ZLEAD + SROWS * s1tot + 1
    return dict(
        pair=pair, half=half, cap1=cap1, off1=off1, s1tot=s1tot,
        strip=strip, bank=bank, cap3=cap3,
        obj_id=obj_id, pbase=pbase, nobj=nobj, rtot=rtot,
    )


def _host_prep(xyz, n_photons, coeffs, inv_voxel_size, psf_center):
    x_idx, y_idx, z_idx, series = _geom(xyz, inv_voxel_size, psf_center)
    m = _build_meta(x_idx, y_idx, z_idx)
    pair, half = m["pair"], m["half"]
    cap1, off1, s1tot = m["cap1"], m["off1"], m["s1tot"]
    cap3, obj_id, pbase, nobj = m["cap3"], m["obj_id"], m["pbase"], m["nobj"]
    strip, bank = m["strip"], m["bank"]

    # coeffs table [128=(half,c), NPAIR*676] fp8 e4m3, scaled by 256
    cr = coeffs.reshape(D, HW, C)
    ctab = np.zeros((128, NPAIR * HW), np.float32)
    for p in range(NPAIR):
        ctab[0:64, p * HW : (p + 1) * HW] = cr[ZLO + p].T
        ctab[64:128, p * HW : (p + 1) * HW] = cr[75 + p].T
    ctab = ctab.astype(ml_dtypes.bfloat16)

    lhsT1 = np.zeros((B, 128, NPAIR * S1PAD), np.float32)
    idx = np.zeros((B, 128, nobj), np.int32)
    lhsT3 = np.zeros((B, 128, NT3 * F), np.float32)

    s1slot = np.zeros((B, N), np.int64)  # emitter -> dense DRAM storage slot
    for b in range(B):
        fill1 = np.zeros(NPAIR, np.int64)
        for e in range(N):
            p = pair[b, e]
            s = fill1[p]
            fill1[p] += 1
            s1slot[b, e] = S1CAP * p + s
            hf = half[b, e]
            lhsT1[b, 64 * hf : 64 * hf + C, S1PAD * p + s] = series[b, e]
        fill3 = np.zeros(NT3, np.int64)
        for e in range(N):
            ti = int(strip[b, e]) * NBANK + int(bank[b, e])
            s = fill3[ti]
            fill3[ti] += 1
            assert s < cap3[ti]
            part = int(pbase[ti] + s)
            o = int(obj_id[ti])
            t = int(strip[b, e])
            row = ZLEAD + SROWS * s1slot[b, e] + ((26 * t - 4) - int(y_idx[b, e]))
            x_rel = int(x_idx[b, e]) - BANK_LO[int(bank[b, e])]
            assert 0 <= x_rel < 32, x_rel
            idx[b, part, o] = row * ROWPX + PIECE0 - x_rel
            lhsT3[b, part, ti * F : (ti + 1) * F] = n_photons[b, e]

    lhsT1 = lhsT1.astype(ml_dtypes.bfloat16)
    lhsT3 = lhsT3.astype(ml_dtypes.bfloat16)
    return m, ctab, lhsT1, lhsT3, idx


def _build_bass(meta):
    nc = bacc.Bacc()
    f32 = mybir.dt.float32
    bf16 = mybir.dt.bfloat16
    i32 = mybir.dt.int32

    cap1, off1 = meta["cap1"], meta["off1"]
    s1tot, nobj, rtot = meta["s1tot"], meta["nobj"], meta["rtot"]
    cap3, obj_id, pbase = meta["cap3"], meta["obj_id"], meta["pbase"]

    ctab_d = nc.declare_dram_parameter("ctab", [128, NPAIR * HW], bf16, isOutput=False)
    lhsT1_d = nc.declare_dram_parameter(
        "lhsT1", [128, NPAIR * S1PAD], bf16, isOutput=False
    )
    lhsT3_d = nc.declare_dram_parameter("lhsT3", [128, NT3 * F], bf16, isOutput=False)
    idx_d = nc.declare_dram_parameter("idx", [128, nobj], i32, isOutput=False)
    rend_d = nc.declare_dram_parameter("rend", [rtot, ROWPX], bf16, isOutput=False)
    out_d = nc.declare_dram_parameter("out", [F, IMG * IMG], f32, isOutput=True)
    if DEBUG:
        dbg_st_d = nc.declare_dram_parameter(
            "dbg_st", [NPAIR // 4, 32, 4 * SROWS * ROWPX], bf16, isOutput=True
        )
        dbg_g_d = nc.declare_dram_parameter(
            "dbg_g", [meta["nobj"], 128, WROWS * ROWPX], bf16, isOutput=True
        )
        dbg_r_d = nc.declare_dram_parameter(
            "dbg_r", [NPAIR // 4, 72, SROWS * ROWPX], bf16, isOutput=True
        )

    with tile.TileContext(nc) as tc:
        with (
            tc.tile_pool(name="ctabp", bufs=1) as ctp,
            tc.tile_pool(name="small", bufs=1) as sp,
            tc.tile_pool(name="rp", bufs=1) as rp,
            tc.tile_pool(name="stp", bufs=2) as stp,
            tc.tile_pool(name="ostg", bufs=4) as op_,
        ):
            rend = rend_d
            zl = sp.tile([128, 128], bf16)
            nc.vector.memset(zl[:], 0.0)
            l1 = sp.tile([128, NPAIR * S1PAD], bf16)
            nc.sync.dma_start(l1[:], lhsT1_d[:])
            l3 = sp.tile([128, NT3 * F], bf16)
            nc.sync.dma_start(l3[:], lhsT3_d[:])
            idxt = sp.tile([128, nobj], i32)
            nc.sync.dma_start(idxt[:], idx_d[:])

            # coeffs table on the ACT HWDGE queue, chunked (separate tiles
            # so stage-1 groups start as soon as their chunk lands)
            ctabs = []
            for j in range(4):
                p0, p1 = 16 * j, min(16 * (j + 1), NPAIR)
                ct = ctp.tile(
                    [128, (p1 - p0) * HW], bf16, tag=f"ctab{j}", name=f"ctab{j}"
                )
                nc.scalar.dma_start(ct[:], ctab_d[:, p0 * HW : p1 * HW])
                ctabs.append(ct)

            def ctab_slice(p, c0, c1):
                return ctabs[p // 16][:, (p % 16) * HW + c0 : (p % 16) * HW + c1]

            # ---- stage 1: rendered patches into padded DRAM storage ----
            with tc.tile_pool(name="ps1", bufs=3, space="PSUM") as pp1:
                for gi in range(NPAIR // 4):
                    ps = pp1.tile([128, HW], f32, tag="ps")
                    for r in range(4):
                        p = 4 * gi + r
                        base = 32 * r
                        lslice = l1[:, p * S1PAD : (p + 1) * S1PAD]
                        nc.tensor.matmul(
                            ps[base : base + S1PAD, 0:512],
                            lhsT=lslice,
                            rhs=ctab_slice(p, 0, 512),
                            start=True, stop=True,
                            tile_position=(0, base),
                        )
                        nc.tensor.matmul(
                            ps[base : base + S1PAD, 512:HW],
                            lhsT=lslice,
                            rhs=ctab_slice(p, 512, HW),
                            start=True, stop=True,
                            tile_position=(0, base),
                        )
                    st = stp.tile([128, PH * ROWPX], bf16, tag="st", bufs=3)
                    if gi < 3:
                        nc.vector.memset(st[:], 0.0)
                    st3 = st[:].rearrange("p (r c) -> p r c", r=PH)
                    eng = nc.scalar if gi % 2 else nc.vector
                    eng_copy = eng.copy if hasattr(eng, "copy") else eng.tensor_copy
                    eng_copy(
                        out=st3[:, 0:PH, PIECE0 : PIECE0 + PW],
                        in_=ps[:, :].rearrange("p (h w) -> p h w", h=PH),
                    )
                    g_slots = 4 * S1CAP  # 128 slots per group
                    nc.sync.dma_start(
                        rend[
                            ZLEAD + SROWS * g_slots * gi : ZLEAD + SROWS * g_slots * (gi + 1), :
                        ].rearrange("(s r) c -> s r c", r=SROWS)[:, 0:PH, :],
                        st[:].rearrange("p (r c) -> p r c", r=PH),
                    )

            # ---- gathers: placed patches -> per-(strip,bank) slot tiles ----
            gobjs = []
            for o in range(nobj):
                g = rp.tile([128, WROWS * ROWPX], bf16, tag=f"g{o}", name=f"g{o}")
                nc.gpsimd.indirect_dma_start(
                    out=g[:],
                    out_offset=None,
                    in_=rend[:],
                    in_offset=bass.IndirectOffsetOnAxis(ap=idxt[:, o : o + 1], axis=1),
                )
                if DEBUG:
                    nc.sync.dma_start(dbg_g_d[o], g[:])
                gobjs.append(g)

            # ---- stage 3: photon expansion, 4 output blocks per PSUM bank ----
            with tc.tile_pool(name="ps3", bufs=4, space="PSUM") as pp3:
                for jt in range(8):  # 8 psum tiles x 4 blocks = 32 blocks
                    cs = pp3.tile([128, 512], f32, tag="cs")
                    # clear has_written + zero the bank
                    nc.tensor.matmul(
                        cs[:, 0:512], lhsT=zl[:], rhs=ctabs[0][:, 0:512],
                        start=True, stop=False, skip_group_check=True,
                    )
                    mms = []
                    for bi in range(4):
                        j = 4 * jt + bi
                        for t in range(NSTRIP):
                            r0 = 30 + 4 * j - 26 * t
                            if 0 <= r0 and r0 + 4 <= WROWS:
                                for k in range(NBANK):
                                    mms.append((bi, t, r0, k))
                    for si, (bi, t, r0, k) in enumerate(mms):
                        j = 4 * jt + bi
                        lo = max(0, BANK_LO[k] - 26)
                        hi = min(IMG, BANK_LO[k] + 31)  # BANK_LO+57-26
                        ti = t * NBANK + k
                        g3 = gobjs[int(obj_id[ti])][:].rearrange(
                            "p (r c) -> p r c", r=WROWS
                        )
                        cs3 = cs[32 * bi : 32 * bi + 16, :].rearrange(
                            "f (r c) -> f r c", c=IMG
                        )
                        nc.tensor.matmul(
                            cs3[:, 0:4, lo:hi],
                            lhsT=l3[:, ti * F : (ti + 1) * F],
                            rhs=g3[:, r0 : r0 + 4, lo + 26 - BANK_LO[k] : hi + 26 - BANK_LO[k]],
                            start=False,
                            stop=(si == len(mms) - 1),
                            skip_group_check=True,
                            tile_position=(0, 32 * bi),
                        )
                    ot = op_.tile([16, 4 * 4 * IMG], f32, tag="ot")
                    for bi in range(4):
                        ceng = nc.scalar if bi % 2 else nc.vector
                        ceng_copy = ceng.copy if hasattr(ceng, "copy") else ceng.tensor_copy
                        ceng_copy(
                            out=ot[:, bi * 4 * IMG : (bi + 1) * 4 * IMG],
                            in_=cs[32 * bi : 32 * bi + 16, :],
                        )
                    oeng = nc.sync if jt % 2 else nc.scalar
                    oeng.dma_start(
                        out_d[:, jt * 4 * 4 * IMG : (jt + 1) * 4 * 4 * IMG],
                        ot[:],
                    )
    if not nc.is_finalized():
        nc.finalize()
    return nc


def prepare(xyz, n_photons, coeffs, inv_voxel_size, psf_center):
    global _compiled, _meta_key
    xyz = np.asarray(xyz, dtype=np.float32)
    n_photons = np.asarray(n_photons, dtype=np.float32)
    coeffs = np.asarray(coeffs, dtype=np.float32)
    inv_voxel_size = np.asarray(inv_voxel_size, dtype=np.float32)
    psf_center = np.asarray(psf_center, dtype=np.float32)

    meta, ctab, lhsT1, lhsT3, idx = _host_prep(
        xyz, n_photons, coeffs, inv_voxel_size, psf_center
    )
    key = (meta["s1tot"], meta["nobj"], tuple(meta["cap1"]), tuple(meta["cap3"]))
    if _compiled is None or _meta_key != key:
        _compiled = _build_bass(meta)
        _meta_key = key
    global _rend_zeros
    if _rend_zeros is None or _rend_zeros.shape[0] != meta["rtot"]:
        _rend_zeros = np.zeros((meta["rtot"], ROWPX), dtype=ml_dtypes.bfloat16)
    in_maps = [
        {"ctab": ctab, "lhsT1": lhsT1[b], "lhsT3": lhsT3[b], "idx": idx[b],
         "rend": _rend_zeros}
        for b in range(B)
    ]
    return _compiled, in_maps


def kernel(xyz, n_photons, coeffs, inv_voxel_size, psf_center, img_size):
    nc, in_maps = prepare(xyz, n_photons, coeffs, inv_voxel_size, psf_center)
    res = run_bass_kernel_spmd(nc, in_maps, core_ids=list(range(B)))
    out = np.stack(
        [res.results[b]["out"].reshape(F, IMG, IMG) for b in range(B)], axis=0
    )
    return out
